# revision 1
# baseline (speedup 1.0000x reference)
"""Multi-head cross-attention on Trainium2, 8-core SPMD.

Problem (hardcoded): B=4, T=2048, D=1024, H=16 heads, head_dim=64, fp32.
    kv = x_enc @ Wkv + bkv ; q = x_dec @ Wq + bq
    per head: S = q_h k_h^T / sqrt(64); P = softmax(S + mask); O_h = P v_h
    out = concat_h(O_h) @ Wo + bo

Sharding: data parallel over batch (4 slices x 2 cores each) and tensor
parallel over heads within each pair (8 heads per core).  Each core
computes a partial output  Y_local @ Wo[rows_local]  (+bo on the even
core of the pair); the host unshards by summing each pair's partials
(the reduce step for the row-sharded output projection) and stacking
the 4 batch slices.  Host-side shard prep also pre-transposes the two
activation matrices (channels-major layout) and regroups Wkv columns
so every device-side access is a plain slice.

The mask input is structurally zero for this problem (spec fill
"zeros"); softmax(S + 0) == softmax(S), so the kernel does not load it
(checked on the host).

Per-core pipeline (transposed layouts throughout; f32r = fast fp32):
  A: K^T = Wkv_K^T x_enc^T (head-pair-stacked), V^T -> V natural (PE
     transpose, with a ones column appended per k-tile)
  B: Q^T = Wq^T x_dec^T, written as two parity copies with the other
     parity's partitions zeroed (so S^T matmuls contract over 128)
  C: per (q-chunk, head-pair, parity): S^T = (K^T tile)^T Q^T_parity;
     P^T = exp(S^T/8) on ACT; O'^T = V_aug^T P^T accumulated in PSUM
     (ones column gives the softmax denominator); normalize via DVE
     mult with PE-broadcast approx-reciprocal of the sums.
  D: out[q,:] = (Y^T as stationary)^T @ Wo tiles, +bo, contiguous DMA.
"""

import numpy as np

import concourse.bass as bass
import concourse.mybir as mybir
import concourse.tile as tile
from concourse import bacc
from concourse.bass_utils import run_bass_kernel_spmd
from concourse.masks import make_identity

f32 = mybir.dt.float32
f32r = mybir.dt.float32r
AF = mybir.ActivationFunctionType
ALU = mybir.AluOpType

P = 128


def build_nc(T=2048, D=1024, HPC=8, HD=64, n_cores=8, use_f32r=True,
             debug=False):
    """Build + compile the per-core Bass program. HPC = heads per core."""
    assert HD == 64 and HPC % 2 == 0 and T % 512 == 0 and D % P == 0
    CPC = HPC * HD          # q/out channels per core
    TC = 512                # token chunk (psum free dim), phases A/B
    QC = 512                # q chunk, attention phase
    NQ = T // TC            # token chunks
    ND = D // P             # model-dim chunks
    NG = HPC // 2           # head pairs
    NKT = T // P            # k-token tiles
    HD1 = HD + 1            # V columns + ones column
    SCALE = float(1.0 / np.sqrt(HD))
    ON = min(512, D)        # out-proj free chunk
    NON = D // ON

    MDT = f32r if use_f32r else f32    # dtype of matmul-input tiles

    def rr_(ap):                       # bitcast for DMA loads from f32 DRAM
        return ap.bitcast(MDT) if use_f32r else ap

    nc = bacc.Bacc("TRN2", target_bir_lowering=False, debug=False,
                   enable_asserts=False, num_devices=n_cores)

    xeT = nc.dram_tensor("x_enc_t", [D, T], f32, kind="ExternalInput").ap()
    xdT = nc.dram_tensor("x_dec_t", [D, T], f32, kind="ExternalInput").ap()
    wq_d = nc.dram_tensor("wq", [D, CPC], f32, kind="ExternalInput").ap()
    wkv_d = nc.dram_tensor("wkv_g", [D, 2 * CPC], f32, kind="ExternalInput").ap()
    wo_d = nc.dram_tensor("wo", [CPC, D], f32, kind="ExternalInput").ap()
    bq_d = nc.dram_tensor("bq", [CPC], f32, kind="ExternalInput").ap()
    bkv_d = nc.dram_tensor("bkv_g", [2 * CPC], f32, kind="ExternalInput").ap()
    bo_d = nc.dram_tensor("bo", [D], f32, kind="ExternalInput").ap()
    out_d = nc.dram_tensor("out", [T, D], f32, kind="ExternalOutput").ap()
    dbg = {}
    if debug:
        for nm, shape in [("dbg_kt", [P, T]), ("dbg_qt", [P, T]),
                          ("dbg_vnat", [P, NKT * HD1]),
                          ("dbg_pt", [P, 2 * QC]), ("dbg_poe", [P, QC]),
                          ("dbg_poo", [P, QC]), ("dbg_rbc", [P, QC]),
                          ("dbg_yt", [P, QC])]:
            dbg[nm] = nc.dram_tensor(nm, shape, f32, kind="ExternalOutput").ap()

    with tile.TileContext(nc) as tc:
      with tc.tile_pool(name="const", bufs=1) as cpool:
        ident = cpool.tile([P, P], f32, name="ident")
        make_identity(nc, ident)
        ones_t = cpool.tile([P, P], MDT, name="ones_t")
        nc.vector.tensor_scalar(ones_t[HD:HD + 1, :], ident[HD:HD + 1, :],
                                0.0, 1.0, ALU.mult, ALU.add)
        nc.vector.tensor_scalar(ones_t[0:1, :], ident[0:1, :],
                                0.0, 1.0, ALU.mult, ALU.add)
        bo_row = cpool.tile([1, D], MDT, name="bo_row")
        bkv_sb = cpool.tile([P, 2 * NG], f32, name="bkv_sb")
        bq_sb = cpool.tile([P, NG], f32, name="bq_sb")
        nc.sync.dma_start(out=bo_row[:], in_=rr_(bo_d[:].unsqueeze(0)))
        for g in range(NG):
            nc.sync.dma_start(out=bkv_sb[:, g:g + 1],
                              in_=bkv_d[g * P:(g + 1) * P].unsqueeze(1))
            nc.sync.dma_start(out=bkv_sb[:, NG + g:NG + g + 1],
                              in_=bkv_d[CPC + g * P:CPC + (g + 1) * P].unsqueeze(1))
            nc.sync.dma_start(out=bq_sb[:, g:g + 1],
                              in_=bq_d[g * P:(g + 1) * P].unsqueeze(1))

        # persistent across A->C
        kT = [cpool.tile([P, T], MDT, name=f"kT{g}") for g in range(NG)]
        vnat = [cpool.tile([P, 2 * NKT * HD1], MDT, name=f"vnat{g}")
                for g in range(NG)]     # per pair, head parity h2 in halves

        def vn(h):                      # per-head view [P, NKT*HD1]
            g, h2 = divmod(h, 2)
            off = h2 * NKT * HD1
            return vnat[g][:, off:off + NKT * HD1]

        # ---------------- Phase A: x_enc^T -> kT, vnat ----------------
        with tc.tile_pool(name="phA", bufs=1) as apool, \
             tc.tile_pool(name="phA_ps2", bufs=2, space="PSUM") as aps2, \
             tc.tile_pool(name="phA_ps3", bufs=2, space="PSUM") as aps3:
            wkv_sb = [apool.tile([P, 2 * CPC], MDT, name=f"wkv{d}")
                      for d in range(ND)]
            for d in range(ND):
                nc.sync.dma_start(out=wkv_sb[d][:],
                                  in_=rr_(wkv_d[d * P:(d + 1) * P, :]))

            for tq in range(NQ):
                tcols = slice(tq * TC, (tq + 1) * TC)
                xcs = []
                for d in range(ND):
                    xc = apool.tile([P, TC], MDT, tag=f"xchA{d}", bufs=2,
                                    name=f"xchA_{tq}_{d}")
                    nc.sync.dma_start(
                        out=xc[:], in_=rr_(xeT[d * P:(d + 1) * P, tcols]))
                    xcs.append(xc)
                for g in range(NG):          # K groups
                    pkv = aps2.tile([P, TC], f32, tag="pkv", name=f"pk_{tq}_{g}")
                    for d in range(ND):
                        nc.tensor.matmul(pkv[:],
                                         wkv_sb[d][:, g * P:(g + 1) * P],
                                         xcs[d][:],
                                         start=(d == 0), stop=(d == ND - 1))
                    nc.vector.tensor_scalar_add(
                        kT[g][:, tcols], pkv[:], bkv_sb[:, g:g + 1])
                vts = []
                for g in range(NG):          # V groups
                    pkv = aps2.tile([P, TC], f32, tag="pkv", name=f"pv_{tq}_{g}")
                    for d in range(ND):
                        nc.tensor.matmul(
                            pkv[:],
                            wkv_sb[d][:, CPC + g * P:CPC + (g + 1) * P],
                            xcs[d][:],
                            start=(d == 0), stop=(d == ND - 1))
                    vt = apool.tile([P, TC], f32, tag=f"vT{g}", bufs=2,
                                    name=f"vT_{tq}_{g}")
                    nc.vector.tensor_scalar_add(vt[:], pkv[:],
                                                bkv_sb[:, NG + g:NG + g + 1])
                    vts.append(vt)
                # V^T chunk -> V natural (PE transpose, 64-row tiles)
                for g in range(NG):
                    for h2 in range(2):
                        h = 2 * g + h2
                        pv = aps3.tile([P, 4 * HD], f32, tag="pvT",
                                       name=f"pvT_{tq}_{g}_{h2}")
                        for i in range(4):
                            nc.tensor.transpose(
                                pv[:, i * HD:(i + 1) * HD],
                                vts[g][h2 * HD:(h2 + 1) * HD,
                                       i * P:(i + 1) * P],
                                ident[h2 * HD:(h2 + 1) * HD,
                                      h2 * HD:(h2 + 1) * HD])
                        blk = vn(h)[:, tq * 4 * HD1:(tq * 4 + 4) * HD1] \
                            .rearrange("p (c x) -> p c x", c=4)
                        src = pv[:].rearrange("p (c x) -> p c x", c=4)
                        nc.vector.tensor_copy(blk[:, :, 0:HD], src)
                        nc.vector.tensor_scalar(
                            blk[:, :, HD:HD1], pv[:, 0:4].unsqueeze(2),
                            0.0, 1.0, ALU.mult, ALU.add)

        # pools for B..D (allocated after phase A space is freed)
        with tc.tile_pool(name="late", bufs=1) as lpool:
            qTe = [lpool.tile([P, T], MDT, name=f"qTe{g}") for g in range(NG)]
            qTo = [lpool.tile([P, T], MDT, name=f"qTo{g}") for g in range(NG)]
            wo_sb = [lpool.tile([P, D], MDT, name=f"wo{g}") for g in range(NG)]
            bo_bc = lpool.tile([P, D], f32, name="bo_bc")
            for g in range(NG):
                nc.sync.dma_start(out=wo_sb[g][:],
                                  in_=rr_(wo_d[g * P:(g + 1) * P, :]))

            # ------- Phase B: x_dec^T -> qTe/qTo (zero-padded parities) ------
            with tc.tile_pool(name="phB", bufs=1) as bpool, \
                 tc.tile_pool(name="phB_ps2", bufs=2, space="PSUM") as bps2:
                wq_sb = [bpool.tile([P, CPC], MDT, name=f"wq{d}")
                         for d in range(ND)]
                for d in range(ND):
                    nc.sync.dma_start(out=wq_sb[d][:],
                                      in_=rr_(wq_d[d * P:(d + 1) * P, :]))
                for tq in range(NQ):
                    tcols = slice(tq * TC, (tq + 1) * TC)
                    xcs = []
                    for d in range(ND):
                        xc = bpool.tile([P, TC], MDT, tag=f"xchB{d}", bufs=2,
                                        name=f"xchB_{tq}_{d}")
                        nc.sync.dma_start(
                            out=xc[:], in_=rr_(xdT[d * P:(d + 1) * P, tcols]))
                        xcs.append(xc)
                    for g in range(NG):
                        pq = bps2.tile([P, TC], f32, tag="pq",
                                       name=f"pq_{tq}_{g}")
                        for d in range(ND):
                            nc.tensor.matmul(pq[:],
                                             wq_sb[d][:, g * P:(g + 1) * P],
                                             xcs[d][:],
                                             start=(d == 0), stop=(d == ND - 1))
                        nc.vector.tensor_scalar_add(
                            qTe[g][0:HD, tcols], pq[0:HD, :],
                            bq_sb[0:HD, g:g + 1])
                        nc.vector.tensor_scalar(
                            qTe[g][HD:P, tcols], pq[HD:P, :], 0.0, 0.0,
                            ALU.mult, ALU.add)
                        nc.vector.tensor_scalar_add(
                            qTo[g][HD:P, tcols], pq[HD:P, :],
                            bq_sb[HD:P, g:g + 1])
                        nc.vector.tensor_scalar(
                            qTo[g][0:HD, tcols], pq[0:HD, :], 0.0, 0.0,
                            ALU.mult, ALU.add)

            # ------------- Phase C/D: attention + out-projection -------------
            with tc.tile_pool(name="phC", bufs=1) as cp2, \
                 tc.tile_pool(name="phC_psS", bufs=2, space="PSUM") as psS, \
                 tc.tile_pool(name="phC_psO", bufs=2, space="PSUM") as psO, \
                 tc.tile_pool(name="phD_ps", bufs=2, space="PSUM") as psD:
                # bo broadcast across partitions via PE ones-matmul
                for o in range(0, D, 512):
                    ow = min(512, D - o)
                    ps_bo = psD.tile([P, 512], f32, tag="pout",
                                     name=f"psbo_{o}")
                    nc.tensor.matmul(ps_bo[:, 0:ow], ones_t[0:1, :],
                                     bo_row[0:1, o:o + ow],
                                     skip_group_check=True)
                    nc.vector.tensor_copy(bo_bc[:, o:o + ow], ps_bo[:, 0:ow])
                if debug:
                    nc.sync.dma_start(out=dbg["dbg_kt"],
                                      in_=kT[0][:].bitcast(f32))
                    nc.sync.dma_start(out=dbg["dbg_qt"],
                                      in_=qTe[0][:].bitcast(f32))
                    nc.sync.dma_start(out=dbg["dbg_vnat"],
                                      in_=vn(0).bitcast(f32))

                def outproj(qc, yTs):
                    for qt in range(QC // P):
                        ost = cp2.tile([P, D], f32, tag="ost", bufs=2,
                                       name=f"ost_{qc}_{qt}")
                        for o in range(NON):
                            ocols = slice(o * ON, (o + 1) * ON)
                            pout = psD.tile([P, ON], f32, tag="pout",
                                            name=f"pout_{qc}_{qt}_{o}")
                            for g in range(NG):
                                nc.tensor.matmul(
                                    pout[:],
                                    yTs[g][:, qt * P:(qt + 1) * P],
                                    wo_sb[g][:, ocols],
                                    start=(g == 0), stop=(g == NG - 1),
                                    skip_group_check=True)
                            nc.vector.tensor_add(ost[:, ocols], pout[:],
                                                 bo_bc[:, ocols])
                        row0 = (qc * (QC // P) + qt) * P
                        nc.sync.dma_start(out=out_d[row0:row0 + P, :],
                                          in_=ost[:])

                NQC = T // QC
                prev = None
                for qc in range(NQC):
                    qcols = slice(qc * QC, (qc + 1) * QC)
                    yTs = [cp2.tile([P, QC], MDT, tag=f"yT{g}", bufs=2,
                                    name=f"yT_{qc}_{g}") for g in range(NG)]
                    for g in range(NG):
                        po = [psO.tile([P, QC], f32, tag="po",
                                       name=f"po_{qc}_{g}_{h2}")
                              for h2 in range(2)]
                        pss = {}
                        pts = {}

                        def mm1(kt, qc=qc, g=g, pss=pss):
                            ps = psS.tile([P, 2 * QC], f32, tag="ps",
                                          name=f"ps_{qc}_{g}_{kt}")
                            for h2, qT in ((0, qTe), (1, qTo)):
                                nc.tensor.matmul(
                                    ps[:, h2 * QC:(h2 + 1) * QC],
                                    kT[g][:, kt * P:(kt + 1) * P],
                                    qT[g][:, qcols],
                                    skip_group_check=True)
                            pss[kt] = ps

                        def do_exp(kt, qc=qc, g=g, pss=pss, pts=pts):
                            pt = cp2.tile([P, 2 * QC], MDT, tag="pt", bufs=3,
                                          name=f"pt_{qc}_{g}_{kt}")
                            nc.scalar.activation(pt[:], pss[kt][:], AF.Exp,
                                                 scale=SCALE)
                            if debug and qc == 0 and g == 0 and kt == 0:
                                nc.sync.dma_start(out=dbg["dbg_pt"],
                                                  in_=pt[:].bitcast(f32))
                            pts[kt] = pt

                        def mm2(kt, g=g, po=po, pts=pts):
                            pt = pts[kt]
                            for h2 in range(2):
                                nc.tensor.matmul(
                                    po[h2][0:HD1, :],
                                    vn(2 * g + h2)[:, kt * HD1:(kt + 1) * HD1],
                                    pt[:, h2 * QC:(h2 + 1) * QC],
                                    start=(kt == 0), stop=(kt == NKT - 1),
                                    skip_group_check=True)

                        # software pipeline over kt
                        mm1(0)
                        mm1(1)
                        do_exp(0)
                        do_exp(1)
                        for k2 in range(1, NKT // 2):
                            mm1(2 * k2)
                            mm1(2 * k2 + 1)
                            mm2(2 * k2 - 2)
                            mm2(2 * k2 - 1)
                            do_exp(2 * k2)
                            do_exp(2 * k2 + 1)
                        mm2(NKT - 2)
                        mm2(NKT - 1)

                        if debug and qc == 0 and g == 0:
                            for h2, nm in ((0, "dbg_poe"), (1, "dbg_poo")):
                                dstg = cp2.tile([P, QC], f32, tag="dstg",
                                                bufs=2,
                                                name=f"dstg_{qc}_{g}_{h2}")
                                nc.vector.tensor_copy(dstg[:], po[h2][:])
                                nc.sync.dma_start(out=dbg[nm], in_=dstg[:])

                        # normalize + evict to yT; the two parity chains
                        # are interleaved stage-by-stage so DVE/PE pipeline
                        # and the po accumulators release sooner
                        srow = [cp2.tile([P, QC], MDT, tag="srow", bufs=2,
                                         name=f"srow_{qc}_{g}_{h2}")
                                for h2 in range(2)]
                        rbc = [cp2.tile([P, QC], f32, tag="rbc", bufs=2,
                                        name=f"rbc_{qc}_{g}_{h2}")
                               for h2 in range(2)]
                        scr = [cp2.tile([P, QC], f32, tag="scr", bufs=2,
                                        name=f"scr_{qc}_{g}_{h2}")
                               for h2 in range(2)]
                        ps_bc = [psD.tile([P, QC], f32, tag="pout",
                                          name=f"psbc_{qc}_{g}_{h2}")
                                 for h2 in range(2)]
                        for h2 in range(2):
                            nc.vector.tensor_copy(srow[h2][HD:HD1, :],
                                                  po[h2][HD:HD1, :])
                        for h2 in range(2):
                            nc.tensor.matmul(ps_bc[h2][0:HD, :],
                                             ones_t[HD:HD1, 0:HD],
                                             srow[h2][HD:HD1, :],
                                             skip_group_check=True)
                        for h2 in range(2):
                            nc.vector.reciprocal_approx_accurate(
                                out=rbc[h2][0:HD, :], in_=ps_bc[h2][0:HD, :],
                                scratch=scr[h2][0:HD, :])
                        if debug and qc == 0 and g == 0:
                            nc.sync.dma_start(out=dbg["dbg_rbc"],
                                              in_=rbc[0][:])
                        nc.vector.tensor_mul(yTs[g][0:HD, :],
                                             po[0][0:HD, :], rbc[0][0:HD, :])
                        stg = cp2.tile([P, QC], MDT, tag="stg",
                                       bufs=2, name=f"stg_{qc}_{g}")
                        nc.vector.tensor_mul(stg[0:HD, :],
                                             po[1][0:HD, :], rbc[1][0:HD, :])
                        nc.sync.dma_start(out=yTs[g][HD:P, :],
                                          in_=stg[0:HD, :])
                    if debug and qc == 0:
                        nc.sync.dma_start(out=dbg["dbg_yt"],
                                          in_=yTs[0][:].bitcast(f32))

                    # out-projection deferred one chunk for PE overlap
                    if prev is not None:
                        outproj(prev[0], prev[1])
                    prev = (qc, yTs)
                if prev is not None:
                    outproj(prev[0], prev[1])

    nc.compile()
    return nc


# ---------------------------------------------------------------------------
# Host side: sharding, run, unshard
# ---------------------------------------------------------------------------

_NC_CACHE = {}


def _get_nc():
    key = "full"
    if key not in _NC_CACHE:
        _NC_CACHE[key] = build_nc()
    return _NC_CACHE[key]


def _group_kv_cols(w_slice, HPC, HD):
    """Reorder kv columns [h-major, (k|v), d] -> K head-pair groups then V."""
    last = w_slice.shape[-1]
    assert last == HPC * 2 * HD
    arr = w_slice.reshape(w_slice.shape[:-1] + (HPC, 2, HD))
    kpart = arr[..., :, 0, :].reshape(w_slice.shape[:-1] + (HPC * HD,))
    vpart = arr[..., :, 1, :].reshape(w_slice.shape[:-1] + (HPC * HD,))
    return np.ascontiguousarray(np.concatenate([kpart, vpart], axis=-1))


def make_in_maps(x_enc, x_dec, Wq, bq, Wkv, bkv, Wo, bo, n_cores=8,
                 HPC=8, HD=64):
    CPC = HPC * HD
    in_maps = []
    xet = [np.ascontiguousarray(x_enc[b].T) for b in range(x_enc.shape[0])]
    xdt = [np.ascontiguousarray(x_dec[b].T) for b in range(x_dec.shape[0])]
    for c in range(n_cores):
        b, hg = c // 2, c % 2
        wkv_slice = Wkv[:, hg * 2 * CPC:(hg + 1) * 2 * CPC]
        bkv_slice = bkv[hg * 2 * CPC:(hg + 1) * 2 * CPC]
        in_maps.append({
            "x_enc_t": xet[b],
            "x_dec_t": xdt[b],
            "wq": np.ascontiguousarray(Wq[:, hg * CPC:(hg + 1) * CPC]),
            "wkv_g": _group_kv_cols(wkv_slice, HPC, HD),
            "wo": np.ascontiguousarray(Wo[hg * CPC:(hg + 1) * CPC, :]),
            "bq": np.ascontiguousarray(bq[hg * CPC:(hg + 1) * CPC]),
            "bkv_g": _group_kv_cols(bkv_slice, HPC, HD),
            "bo": np.ascontiguousarray(bo) if hg == 0 else np.zeros_like(bo),
        })
    return in_maps


def kernel(x_enc, x_dec, mask, Wq, bq, Wkv, bkv, Wo, bo):
    x_enc = np.asarray(x_enc, dtype=np.float32)
    x_dec = np.asarray(x_dec, dtype=np.float32)
    Wq = np.asarray(Wq, dtype=np.float32)
    bq = np.asarray(bq, dtype=np.float32)
    Wkv = np.asarray(Wkv, dtype=np.float32)
    bkv = np.asarray(bkv, dtype=np.float32)
    Wo = np.asarray(Wo, dtype=np.float32)
    bo = np.asarray(bo, dtype=np.float32)
    mask = np.asarray(mask)
    if mask.any():
        raise ValueError("kernel assumes a zero additive mask (spec fill=zeros)")

    nc = _get_nc()
    in_maps = make_in_maps(x_enc, x_dec, Wq, bq, Wkv, bkv, Wo, bo)
    res = run_bass_kernel_spmd(nc, in_maps, core_ids=list(range(8)))
    outs = [res.results[c]["out"] for c in range(8)]
    B = x_enc.shape[0]
    full = np.stack([outs[2 * b] + outs[2 * b + 1] for b in range(B)], axis=0)
    return full


if __name__ == "__main__":
    import time
    t0 = time.time()
    nc = _get_nc()
    print(f"build+compile ok in {time.time() - t0:.1f}s")



# revision 9
# speedup vs baseline: 1.0943x; 1.0943x over previous
"""Multi-head cross-attention on Trainium2, 8-core SPMD.

Problem (hardcoded): B=4, T=2048, D=1024, H=16 heads, head_dim=64, fp32.
    kv = x_enc @ Wkv + bkv ; q = x_dec @ Wq + bq
    per head: S = q_h k_h^T / sqrt(64); P = softmax(S + mask); O_h = P v_h
    out = concat_h(O_h) @ Wo + bo

Sharding: data parallel over batch (4 slices x 2 cores each) and tensor
parallel over heads within each pair (8 heads per core).  Each core
computes a partial output  Y_local @ Wo[rows_local]  (+bo on the even
core of the pair); the host unshards by summing each pair's partials
and stacking the 4 batch slices.  Host-side shard prep pre-transposes
the two activation matrices and regroups Wkv columns so every
device-side access is a plain slice.

The mask input is structurally zero for this problem (spec fill
"zeros"); softmax(S + 0) == softmax(S), so the kernel does not load it
(checked on the host).

Per-core pipeline (f32r = fast fp32 everywhere on the PE):
  A: K^T[g] = (Wkv_K cols)^T x_enc^T per head pair (contraction over
     model dim, 128-chunks); V in NATURAL [k, d] layout directly
     (stationary = x_enc^T chunks, moving = Wkv_V columns), written
     into a persistent per-head [k, 65] layout whose 65th column is a
     constant 1 (gives softmax denominators for free in mm2).
  B: Q^T[g] = Wq^T x_dec^T, unpadded pair-stacked [128, T] tiles.
  C: ONE flat software pipeline over all (q-chunk, pair, k-tile)
     steps: S^T tile = (K^T slice)^T Q^T slice per parity
     (64-partition contraction); P^T = exp(S^T/8) on ACT ([128,1024]
     per instruction); O'^T accumulated per parity in PSUM ([65,512],
     65th row = denominator).  At each pair boundary the PSUM
     accumulators are immediately copied to SBUF scratch (fast free),
     then normalized out-of-line: reciprocal of the denominator row
     (DVE approx), PE ones-matmul broadcast to 64 partitions, DVE
     multiply into the pair-stacked Y^T tile (odd parity shifted via
     SBUF->SBUF DMA).  The PREVIOUS q-chunk's output projection
     (Y^T as stationary vs Wo tiles, +bo) is interleaved into the kt
     loop as PE filler so the tensor engine never idles while ACT
     runs exp.
"""

import numpy as np

import concourse.bass as bass
import concourse.mybir as mybir
import concourse.tile as tile
from concourse import bacc
from concourse.bass_utils import run_bass_kernel_spmd
from concourse.masks import make_identity

f32 = mybir.dt.float32
f32r = mybir.dt.float32r
AF = mybir.ActivationFunctionType
ALU = mybir.AluOpType

P = 128


def build_nc(T=2048, D=1024, HPC=8, HD=64, n_cores=8, use_f32r=True):
    """Build + compile the per-core Bass program. HPC = heads per core."""
    assert HD == 64 and HPC % 2 == 0 and T % 512 == 0 and D % P == 0
    CPC = HPC * HD          # q/out channels per core (512)
    TC = 512                # token chunk (psum free dim), phases A/B
    QC = 512                # q chunk width, attention phase
    NQ = T // TC            # token chunks (4)
    ND = D // P             # model-dim chunks (8)
    NG = HPC // 2           # head pairs (4)
    NKT = T // P            # k-token tiles (16)
    NQC = T // QC           # q chunks (4)
    HD1 = HD + 1            # V columns + ones column (65)
    VH = NKT * HD1          # per-head vnat columns (1040)
    SCALE = float(1.0 / np.sqrt(HD))
    ON = 512                # out-proj free chunk
    NON = D // ON           # 2

    MDT = f32r if use_f32r else f32    # dtype of matmul-input tiles

    def rr_(ap):                       # bitcast for DMA loads from f32 DRAM
        return ap.bitcast(MDT) if use_f32r else ap

    nc = bacc.Bacc("TRN2", target_bir_lowering=False, debug=False,
                   enable_asserts=False, num_devices=n_cores)

    xeT = nc.dram_tensor("x_enc_t", [D, T], f32, kind="ExternalInput").ap()
    xdT = nc.dram_tensor("x_dec_t", [D, T], f32, kind="ExternalInput").ap()
    wq_d = nc.dram_tensor("wq", [D, CPC], f32, kind="ExternalInput").ap()
    wkv_d = nc.dram_tensor("wkv_g", [D, 2 * CPC], f32, kind="ExternalInput").ap()
    wo_d = nc.dram_tensor("wo", [CPC, D], f32, kind="ExternalInput").ap()
    bq_d = nc.dram_tensor("bq", [CPC], f32, kind="ExternalInput").ap()
    bkv_d = nc.dram_tensor("bkv_g", [2 * CPC], f32, kind="ExternalInput").ap()
    bo_d = nc.dram_tensor("bo", [D], f32, kind="ExternalInput").ap()
    out_d = nc.dram_tensor("out", [T, D], f32, kind="ExternalOutput").ap()

    with tile.TileContext(nc) as tc:
      with tc.tile_pool(name="const", bufs=1) as cpool:
        # identity (ones source) + ones rows for PE broadcasts: row 0
        # (bias rows at partition 0); f32r producers must be DVE ops
        ident = cpool.tile([P, P], f32, name="ident")
        make_identity(nc, ident)
        ones_t = cpool.tile([P, P], MDT, name="ones_t")
        nc.vector.tensor_scalar(ones_t[0:1, :], ident[0:1, :],
                                0.0, 1.0, ALU.mult, ALU.add)

        bo_row = cpool.tile([1, D], MDT, name="bo_row")
        bkv_k_sb = cpool.tile([P, NG], f32, name="bkv_k_sb")
        bq_sb = cpool.tile([P, NG], f32, name="bq_sb")
        bkv_v_row = cpool.tile([1, CPC], MDT, name="bkv_v_row")
        nc.sync.dma_start(out=bo_row[:], in_=rr_(bo_d[:].unsqueeze(0)))
        nc.sync.dma_start(out=bkv_v_row[:],
                          in_=rr_(bkv_d[CPC:2 * CPC].unsqueeze(0)))
        for g in range(NG):
            nc.sync.dma_start(out=bkv_k_sb[:, g:g + 1],
                              in_=bkv_d[g * P:(g + 1) * P].unsqueeze(1))
            nc.sync.dma_start(out=bq_sb[:, g:g + 1],
                              in_=bq_d[g * P:(g + 1) * P].unsqueeze(1))

        # persistent activations
        kT = [cpool.tile([P, T], MDT, name=f"kT{g}") for g in range(NG)]
        qT = [cpool.tile([P, T], MDT, name=f"qT{g}") for g in range(NG)]
        # V natural, per head h: [k-token partitions, NKT*(HD+1)];
        # column kt*65+64 is constant 1.0 (softmax denominator trick)
        vnat = cpool.tile([P, HPC * VH], MDT, name="vnat")
        vnat3 = vnat[:].rearrange("p (h c) -> p h c", h=HPC)
        nc.vector.tensor_scalar(
            vnat[:].rearrange("p (h k c) -> p h k c", h=HPC, k=NKT)
            [:, :, :, HD:HD1],
            ident[:, 0:HPC * NKT].rearrange("p (h k c) -> p h k c",
                                            h=HPC, k=NKT),
            0.0, 1.0, ALU.mult, ALU.add)

        def vn(h):                      # per-head view [P, VH]
            return vnat3[:, h, :]

        wo_sb = [cpool.tile([P, D], MDT, name=f"wo{g}") for g in range(NG)]
        bo_bc = cpool.tile([P, D], f32, name="bo_bc")
        for g in range(NG):
            nc.sync.dma_start(out=wo_sb[g][:],
                              in_=rr_(wo_d[g * P:(g + 1) * P, :]))

        # ---------------- Phases A+B: projections ----------------
        with tc.tile_pool(name="phAB", bufs=1) as apool, \
             tc.tile_pool(name="phAB_ps", bufs=1, space="PSUM") as aps:
            wkv_sb = [apool.tile([P, 2 * CPC], MDT, name=f"wkv{d}")
                      for d in range(ND)]
            wq_sb = [apool.tile([P, CPC], MDT, name=f"wq{d}")
                     for d in range(ND)]
            for d in range(ND):
                nc.sync.dma_start(out=wkv_sb[d][:],
                                  in_=rr_(wkv_d[d * P:(d + 1) * P, :]))
                nc.sync.dma_start(out=wq_sb[d][:],
                                  in_=rr_(wq_d[d * P:(d + 1) * P, :]))

            # V-bias broadcast row -> [128, 512] (PE ones-matmul)
            bias_vbc = apool.tile([P, CPC], f32, name="bias_vbc")
            ps_vb = aps.tile([P, CPC], f32, tag="pv", bufs=2, name="ps_vb")
            nc.tensor.matmul(ps_vb[:], ones_t[0:1, :], bkv_v_row[0:1, :],
                             skip_group_check=True)
            nc.vector.tensor_copy(bias_vbc[:], ps_vb[:])

            # ---- Phase A: x_enc^T -> kT (pair-stacked), vnat (natural) ----
            for tq in range(NQ):
                tcols = slice(tq * TC, (tq + 1) * TC)
                xcs = []
                for d in range(ND):
                    xc = apool.tile([P, TC], MDT, tag=f"xch{d}", bufs=2,
                                    name=f"xchA_{tq}_{d}")
                    nc.sync.dma_start(
                        out=xc[:], in_=rr_(xeT[d * P:(d + 1) * P, tcols]))
                    xcs.append(xc)
                for g in range(NG):          # K groups
                    pkv = aps.tile([P, TC], f32, tag="pkv", bufs=2,
                                   name=f"pk_{tq}_{g}")
                    for d in range(ND):
                        nc.tensor.matmul(pkv[:],
                                         wkv_sb[d][:, g * P:(g + 1) * P],
                                         xcs[d][:],
                                         start=(d == 0), stop=(d == ND - 1))
                    nc.vector.tensor_scalar_add(
                        kT[g][:, tcols], pkv[:], bkv_k_sb[:, g:g + 1])
                # V natural: per 128-token tile, stationary = x_enc^T chunk
                for i in range(TC // P):
                    kt = tq * (TC // P) + i
                    pv = aps.tile([P, CPC], f32, tag="pv", bufs=2,
                                  name=f"pv_{tq}_{i}")
                    for d in range(ND):
                        nc.tensor.matmul(
                            pv[:], xcs[d][:, i * P:(i + 1) * P],
                            wkv_sb[d][:, CPC:2 * CPC],
                            start=(d == 0), stop=(d == ND - 1))
                    # scatter per-head 64-col blocks into vnat (+ bias)
                    out3 = vnat[:].rearrange("p (h k c) -> p h k c",
                                             h=HPC, k=NKT)[:, :, kt, 0:HD]
                    nc.vector.tensor_tensor(
                        out3,
                        pv[:].rearrange("p (h c) -> p h c", h=HPC),
                        bias_vbc[:].rearrange("p (h c) -> p h c", h=HPC),
                        ALU.add)

            # ---- Phase B: x_dec^T -> qT (pair-stacked, unpadded) ----
            for tq in range(NQ):
                tcols = slice(tq * TC, (tq + 1) * TC)
                xds = []
                for d in range(ND):
                    xd = apool.tile([P, TC], MDT, tag=f"xch{d}", bufs=2,
                                    name=f"xchB_{tq}_{d}")
                    nc.sync.dma_start(
                        out=xd[:], in_=rr_(xdT[d * P:(d + 1) * P, tcols]))
                    xds.append(xd)
                for g in range(NG):
                    pq = aps.tile([P, TC], f32, tag="pkv", bufs=2,
                                  name=f"pq_{tq}_{g}")
                    for d in range(ND):
                        nc.tensor.matmul(pq[:],
                                         wq_sb[d][:, g * P:(g + 1) * P],
                                         xds[d][:],
                                         start=(d == 0), stop=(d == ND - 1))
                    nc.vector.tensor_scalar_add(
                        qT[g][:, tcols], pq[:], bq_sb[:, g:g + 1])

        # ---------------- Phase C: attention + out-projection ----------------
        with tc.tile_pool(name="phC", bufs=1) as cp, \
             tc.tile_pool(name="phC_psS", bufs=1, space="PSUM") as psS, \
             tc.tile_pool(name="phC_psO", bufs=1, space="PSUM") as psO, \
             tc.tile_pool(name="phC_aux", bufs=1, space="PSUM") as psA:

            # bo broadcast across partitions (PE ones-matmul, once)
            for o in range(NON):
                ocols = slice(o * ON, (o + 1) * ON)
                ps_bo = psA.tile([P, ON], f32, tag="aux", bufs=2,
                                 name=f"psbo_{o}")
                nc.tensor.matmul(ps_bo[:], ones_t[0:1, :],
                                 bo_row[0:1, ocols], skip_group_check=True)
                nc.vector.tensor_copy(bo_bc[:, ocols], ps_bo[:])

            # flat pipeline state
            steps = [(qc, g, kt)
                     for qc in range(NQC) for g in range(NG)
                     for kt in range(NKT)]
            NS = len(steps)
            pss = {}      # step idx -> psS tile
            pts = {}      # step idx -> pt tile
            pos = {}      # (qc, g) -> [po_e, po_o]
            yts = {}      # qc -> list of yT tiles per g
            osts = {}     # (qc, qt) -> ost tile

            def mm1(i):
                qc, g, kt = steps[i]
                qcols = slice(qc * QC, (qc + 1) * QC)
                ps = psS.tile([P, 2 * QC], f32, tag="ps", bufs=2,
                              name=f"ps_{i}")
                for h2 in range(2):
                    nc.tensor.matmul(
                        ps[:, h2 * QC:(h2 + 1) * QC],
                        kT[g][h2 * HD:(h2 + 1) * HD, kt * P:(kt + 1) * P],
                        qT[g][h2 * HD:(h2 + 1) * HD, qcols],
                        skip_group_check=True)
                pss[i] = ps

            def do_exp(i):
                pt = cp.tile([P, 2 * QC], MDT, tag="pt", bufs=4,
                             name=f"pt_{i}")
                nc.scalar.activation(pt[:], pss.pop(i)[:], AF.Exp,
                                     scale=SCALE)
                pts[i] = pt

            def mm2(i):
                qc, g, kt = steps[i]
                if kt == 0:
                    pos[(qc, g)] = [
                        psO.tile([HD1, QC], f32, tag="po", bufs=2,
                                 name=f"po_{qc}_{g}_{h2}")
                        for h2 in range(2)]
                po = pos[(qc, g)]
                pt = pts.pop(i)
                for h2 in range(2):
                    nc.tensor.matmul(
                        po[h2][0:HD1, :],
                        vn(2 * g + h2)[:, kt * HD1:(kt + 1) * HD1],
                        pt[:, h2 * QC:(h2 + 1) * QC],
                        start=(kt == 0), stop=(kt == NKT - 1),
                        skip_group_check=True)

            def normalize(qc, g):
                """Evict+normalize pair (qc, g): frees po fast via SBUF
                scratch copies, then reciprocal/broadcast/mul off the
                critical path."""
                if g == 0:
                    yts[qc] = [cp.tile([P, QC], MDT, tag=f"yT{gg}", bufs=2,
                                       name=f"yT_{qc}_{gg}")
                               for gg in range(NG)]
                po = pos.pop((qc, g))
                scr = [cp.tile([HD1, QC], f32, tag=f"scr{h2}", bufs=2,
                               name=f"scr_{qc}_{g}_{h2}") for h2 in range(2)]
                # fast po eviction (DVE), even parity first
                for h2 in range(2):
                    nc.vector.tensor_copy(scr[h2][:], po[h2][0:HD1, :])
                # GpSimd partition_broadcast reads its source on Q7 core 0
                # (partitions 0-15 only): move the denominator row from
                # partition 64 to partition 0 via a tiny SBUF->SBUF DMA
                den = [cp.tile([1, QC], f32, tag=f"den{h2}", bufs=2,
                               name=f"den_{qc}_{g}_{h2}") for h2 in range(2)]
                rr = [cp.tile([1, QC], f32, tag=f"rr{h2}", bufs=2,
                              name=f"rr_{qc}_{g}_{h2}") for h2 in range(2)]
                for h2 in range(2):
                    nc.sync.dma_start(out=den[h2][0:1, :],
                                      in_=scr[h2][HD:HD1, :])
                for h2 in range(2):
                    nc.vector.reciprocal_approx_fast(
                        out=rr[h2][0:1, :], in_=den[h2][0:1, :])
                # broadcast the reciprocal row to 64 partitions on the (idle)
                # GpSimd engine; keeps the whole chain off PE/PSUM
                rbc = [cp.tile([HD, QC], f32, tag=f"rbc{h2}", bufs=2,
                               name=f"rbc_{qc}_{g}_{h2}") for h2 in range(2)]
                for h2 in range(2):
                    nc.gpsimd.partition_broadcast(
                        rbc[h2][:], rr[h2][0:1, :], channels=HD)
                # even parity -> yT rows 0:64 directly
                nc.vector.tensor_tensor(
                    yts[qc][g][0:HD, :],
                    scr[0][0:HD, :], rbc[0][:], ALU.mult)
                # odd parity -> staging tile, DMA shift to rows 64:128
                stg = cp.tile([HD, QC], MDT, tag="stg", bufs=2,
                              name=f"stg_{qc}_{g}")
                nc.vector.tensor_tensor(
                    stg[:],
                    scr[1][0:HD, :], rbc[1][:], ALU.mult)
                nc.sync.dma_start(out=yts[qc][g][HD:P, :], in_=stg[:])

            def outproj_group(qc, qt, o):
                """One out-projection accumulation group for q-subtile qt,
                output-column chunk o, of q-chunk qc."""
                ocols = slice(o * ON, (o + 1) * ON)
                if o == 0:
                    osts[(qc, qt)] = cp.tile([P, D], f32, tag="ost", bufs=2,
                                             name=f"ost_{qc}_{qt}")
                ost = osts[(qc, qt)]
                pout = psA.tile([P, ON], f32, tag="aux", bufs=2,
                                name=f"pout_{qc}_{qt}_{o}")
                for g in range(NG):
                    nc.tensor.matmul(
                        pout[:], yts[qc][g][:, qt * P:(qt + 1) * P],
                        wo_sb[g][:, ocols],
                        start=(g == 0), stop=(g == NG - 1),
                        skip_group_check=True)
                nc.vector.tensor_add(ost[:, ocols], pout[:], bo_bc[:, ocols])
                if o == NON - 1:
                    row0 = (qc * (QC // P) + qt) * P
                    nc.sync.dma_start(out=out_d[row0:row0 + P, :],
                                      in_=osts.pop((qc, qt))[:])

            # out-proj work queue: for each qc, (QC//P)*NON groups, run
            # during the NEXT qc's attention (2 slots per pair: kt 7, 15)
            op_queue = []

            mm1(0)
            mm1(1)
            do_exp(0)
            for i in range(NS):
                if i + 2 < NS:
                    mm1(i + 2)
                mm2(i)
                if i + 1 < NS:
                    do_exp(i + 1)
                qc, g, kt = steps[i]
                if kt == NKT - 1:
                    normalize(qc, g)
                    if g == NG - 1:
                        for qt in range(QC // P):
                            for o in range(NON):
                                op_queue.append((qc, qt, o))
                if kt == 7 or kt == NKT - 1:
                    if op_queue:
                        outproj_group(*op_queue.pop(0))
            while op_queue:
                outproj_group(*op_queue.pop(0))

    nc.compile()
    return nc


# ---------------------------------------------------------------------------
# Host side: sharding, run, unshard
# ---------------------------------------------------------------------------

_NC_CACHE = {}


def _get_nc():
    key = "full"
    if key not in _NC_CACHE:
        _NC_CACHE[key] = build_nc()
    return _NC_CACHE[key]


def _group_kv_cols(w_slice, HPC, HD):
    """Reorder kv columns [h-major, (k|v), d] -> K head-pair groups then V."""
    last = w_slice.shape[-1]
    assert last == HPC * 2 * HD
    arr = w_slice.reshape(w_slice.shape[:-1] + (HPC, 2, HD))
    kpart = arr[..., :, 0, :].reshape(w_slice.shape[:-1] + (HPC * HD,))
    vpart = arr[..., :, 1, :].reshape(w_slice.shape[:-1] + (HPC * HD,))
    return np.ascontiguousarray(np.concatenate([kpart, vpart], axis=-1))


def make_in_maps(x_enc, x_dec, Wq, bq, Wkv, bkv, Wo, bo, n_cores=8,
                 HPC=8, HD=64):
    CPC = HPC * HD
    in_maps = []
    xet = [np.ascontiguousarray(x_enc[b].T) for b in range(x_enc.shape[0])]
    xdt = [np.ascontiguousarray(x_dec[b].T) for b in range(x_dec.shape[0])]
    for c in range(n_cores):
        b, hg = c // 2, c % 2
        wkv_slice = Wkv[:, hg * 2 * CPC:(hg + 1) * 2 * CPC]
        bkv_slice = bkv[hg * 2 * CPC:(hg + 1) * 2 * CPC]
        in_maps.append({
            "x_enc_t": xet[b],
            "x_dec_t": xdt[b],
            "wq": np.ascontiguousarray(Wq[:, hg * CPC:(hg + 1) * CPC]),
            "wkv_g": _group_kv_cols(wkv_slice, HPC, HD),
            "wo": np.ascontiguousarray(Wo[hg * CPC:(hg + 1) * CPC, :]),
            "bq": np.ascontiguousarray(bq[hg * CPC:(hg + 1) * CPC]),
            "bkv_g": _group_kv_cols(bkv_slice, HPC, HD),
            "bo": np.ascontiguousarray(bo) if hg == 0 else np.zeros_like(bo),
        })
    return in_maps


def kernel(x_enc, x_dec, mask, Wq, bq, Wkv, bkv, Wo, bo):
    x_enc = np.asarray(x_enc, dtype=np.float32)
    x_dec = np.asarray(x_dec, dtype=np.float32)
    Wq = np.asarray(Wq, dtype=np.float32)
    bq = np.asarray(bq, dtype=np.float32)
    Wkv = np.asarray(Wkv, dtype=np.float32)
    bkv = np.asarray(bkv, dtype=np.float32)
    Wo = np.asarray(Wo, dtype=np.float32)
    bo = np.asarray(bo, dtype=np.float32)
    mask = np.asarray(mask)
    if mask.any():
        raise ValueError("kernel assumes a zero additive mask (spec fill=zeros)")

    nc = _get_nc()
    in_maps = make_in_maps(x_enc, x_dec, Wq, bq, Wkv, bkv, Wo, bo)
    res = run_bass_kernel_spmd(nc, in_maps, core_ids=list(range(8)))
    outs = [res.results[c]["out"] for c in range(8)]
    B = x_enc.shape[0]
    full = np.stack([outs[2 * b] + outs[2 * b + 1] for b in range(B)], axis=0)
    return full


if __name__ == "__main__":
    import time
    t0 = time.time()
    nc = _get_nc()
    print(f"build+compile ok in {time.time() - t0:.1f}s")
